# revision 33
# baseline (speedup 1.0000x reference)
"""TRN2 Bass kernel: 16-head attention (B=4, S=2048, HID=1024), fp32 in/out.

Full inputs in, full output out. Sharding: core c handles batch c//2 and
head-half c%2 (8 of 16 heads). Q/K/V projections are column-sharded over
heads; the output projection is row-sharded, each core producing a
partial [SQ, HID] f32 product that the host sums per batch pair.

Query compaction: the reference masks QUERY rows only — every masked row
yields the identical uniform-softmax output (mean of V through the output
projection). The host therefore gathers just the unmasked query rows plus
ONE representative masked row (zeroed via maskf, so it computes the exact
uniform-softmax row on device) into a [SQ=1152, HID] compacted block, and
scatters device rows back afterward (masked rows all get the
representative row). Scores, PV, exp, Q-projection and output projection
shrink by ~44%; K/V projections are untouched (keys are never masked).

All matmul operands are bf16 (PSUM accumulation stays f32). The softmax
mask and 1/sqrt(dh) scale are folded into the Q rows; exp runs on ScalarE
straight out of PSUM; the softmax denominator comes free via a per-head
ones column in V'. The attention inner loop is software-pipelined: scores
for chunk kch+1 are emitted before PV of chunk kch so the PE never sits
behind ScalarE's exp in program order.
"""

from contextlib import ExitStack

import numpy as np

import concourse.bass as bass
import concourse.bacc as bacc
import concourse.mybir as mybir
import concourse.tile as tile
from concourse.bass_utils import run_bass_kernel_spmd

DT = mybir.dt
F32 = DT.float32
BF16 = DT.bfloat16
AF = mybir.ActivationFunctionType
ALU = mybir.AluOpType

# Problem constants (hardcoded per harness contract)
B, S, HID, NH, DH = 4, 2048, 1024, 16, 64
N_CORES = 8

TRACE = False
LAST_RESULTS = [None]

# per-core derived constants
HPG = NH // 2            # 8 heads per core
FG = HPG * DH            # 512 local feature dims
FCG = FG // 128          # 4 feature chunks (head pairs)
IC = HID // 128          # 8 input-dim chunks
NKC = S // 128           # 16 key chunks
SQB = 384                # query block (compacted)
XW = 512                 # K/V projection moving width
WV = HPG * 65            # V' row stride (64 dims + ones col per head)


def build(nc: bass.Bass, nsqb: int):
    SQ = nsqb * SQB      # compacted+padded query count
    dp = nc.declare_dram_parameter
    qT = dp("qT", [HID, SQ], BF16, isOutput=False)
    kT = dp("kT", [HID, S], BF16, isOutput=False)
    vT = dp("vT", [HID, S], BF16, isOutput=False)
    wqT = dp("wqT", [HID, FG], BF16, isOutput=False)
    wkT = dp("wkT", [HID, FG], BF16, isOutput=False)
    wvT = dp("wvT", [HID, FG], BF16, isOutput=False)
    woT = dp("woT", [FG, HID], BF16, isOutput=False)
    bqc = dp("bqc", [128, FG // 128], F32, isOutput=False)
    bkc = dp("bkc", [128, FG // 128], F32, isOutput=False)
    bvr = dp("bvr", [1, FG], BF16, isOutput=False)
    bor = dp("bor", [1, HID], BF16, isOutput=False)
    maskf = dp("maskf", [1, SQ], F32, isOutput=False)
    out = dp("out", [SQ, HID], F32, isOutput=True)

    with tile.TileContext(nc) as tc, ExitStack() as ctx:
        cpool = ctx.enter_context(tc.tile_pool(name="consts", bufs=1))
        bqc_sb = cpool.tile([128, FG // 128], F32, tag="bqc")
        bkc_sb = cpool.tile([128, FG // 128], F32, tag="bkc")
        bv_sb = cpool.tile([1, FG], BF16, tag="bv")
        bo_sb = cpool.tile([1, HID], BF16, tag="bo")
        nc.sync.dma_start(bqc_sb[:], bqc[:])
        nc.sync.dma_start(bkc_sb[:], bkc[:])
        # memset can't target bf16: materialize f32, cast-copy
        ones_f32 = cpool.tile([1, 512], F32, tag="ones32")
        nc.vector.memset(ones_f32[:], 1.0)
        ones_row = cpool.tile([1, 512], BF16, tag="ones")
        nc.vector.tensor_copy(ones_row[:], ones_f32[:])
        NOC = NKC * HPG  # ones-column count in V' (128)
        onesw_f32 = cpool.tile([128, NOC], F32, tag="onesw32")
        nc.vector.memset(onesw_f32[:], 1.0)
        ones_wide = cpool.tile([128, NOC], BF16, tag="onesw")
        nc.vector.tensor_copy(ones_wide[:], onesw_f32[:])
        maskB = cpool.tile([128, SQ], F32, tag="maskB")
        dbounce = ctx.enter_context(tc.tile_pool(name="dbounce", bufs=4, space="DRAM"))

        gpool = ctx.enter_context(tc.tile_pool(name="gstore", bufs=1))
        qt = gpool.tile([128, FCG * SQ], BF16, tag="qt")
        kt = gpool.tile([128, FCG * S], BF16, tag="kt")
        vp = gpool.tile([128, NKC * WV], BF16, tag="vp")
        h_tile = gpool.tile([128, FCG * SQ], BF16, tag="h")
        # wo and wq stay resident: 8KB/partition each in bf16 (their DMAs
        # are emitted after the K-projection loads so the critical first
        # projection isn't queued behind 2MB of weights needed much later)
        wo_sb = gpool.tile([128, FCG * HID], BF16, tag="wo")
        wq_sb = gpool.tile([128, IC * FG], BF16, tag="wq")

        # persistent pools so weight/activation DMAs prefetch across phases
        wpool = ctx.enter_context(tc.tile_pool(name="wgt", bufs=2))
        xpool = ctx.enter_context(tc.tile_pool(name="xin", bufs=3))

        def pe_touch(ppool, ap):
            # 1x1 matmul that absorbs a DMA-queue wait into the PE clock, so
            # real matmuls stay within the 2-sync-wait ISA budget
            pt = ppool.tile([1, 1], F32, tag="pt", bufs=2)
            nc.tensor.matmul(pt[:], ap, ap, start=True, stop=True)

        def load_w(wT, ppool, split=1):
            # split=IC on the critical first load: per-ic pieces land
            # incrementally so the ic-inner matmul loop starts on chunk 0
            # while later chunks are still in flight
            w_sb = wpool.tile([128, IC * FG], BF16, tag="w")
            src = wT[:, :].rearrange("(i p) f -> p i f", p=128)
            dst = w_sb[:].rearrange("p (i f) -> p i f", i=IC)
            step = IC // split
            for s in range(split):
                nc.sync.dma_start(
                    dst[:, s * step:(s + 1) * step],
                    src[:, s * step:(s + 1) * step],
                )
            pe_touch(ppool, w_sb[0:1, 0:2].bitcast(F32))
            return w_sb

        def load_x(xT, rb, ppool=None, split=1, w=XW):
            x_sb = xpool.tile([128, IC * w], BF16, tag="x")
            src = xT[:, rb * w:(rb + 1) * w].rearrange(
                "(i p) c -> p i c", p=128
            )
            dst = x_sb[:].rearrange("p (i c) -> p i c", i=IC)
            step = IC // split
            for s in range(split):
                nc.sync.dma_start(
                    dst[:, s * step:(s + 1) * step],
                    src[:, s * step:(s + 1) * step],
                )
            if ppool is not None:
                pe_touch(ppool, x_sb[0:1, 0:2].bitcast(F32))
            return x_sb

        def qproj_closures(sqb_, pool, tag, touch_pool=None):
            # Q projection for one SQB-query block (mask * 1/sqrt(dh)
            # folded into the rows), split into single-matmul closures so
            # it can interleave into the previous block's attention
            st = {}
            ops = [lambda: st.__setitem__(
                "x", load_x(qT, sqb_, touch_pool, w=SQB))]
            for fcg in range(FCG):
                def mm(ic, fcg=fcg):
                    def run():
                        if ic == 0:
                            # [128, 512] so the tag's ring matches the
                            # out-proj tiles; matmuls use the bank-aligned
                            # first SQB columns
                            st["ps"] = pool.tile([128, 512], F32,
                                                 tag=tag, name=tag)
                        nc.tensor.matmul(
                            st["ps"][:, 0:SQB],
                            wq_sb[:, ic * FG + fcg * 128:
                                  ic * FG + fcg * 128 + 128],
                            st["x"][:, ic * SQB:(ic + 1) * SQB],
                            start=(ic == 0),
                            stop=(ic == IC - 1),
                        )
                    return run

                def fin(fcg=fcg):
                    # qt = (ps + bq) * mask in one DVE op
                    nc.vector.scalar_tensor_tensor(
                        qt[:, fcg * SQ + sqb_ * SQB:
                           fcg * SQ + (sqb_ + 1) * SQB],
                        st["ps"][:, 0:SQB],
                        bqc_sb[:, fcg:fcg + 1],
                        maskB[:, sqb_ * SQB:(sqb_ + 1) * SQB],
                        ALU.add,
                        ALU.mult,
                    )

                ops.extend([mm(ic) for ic in range(IC)])
                ops.append(fin)
            return ops

        # fill the per-head ones columns (col 64 of each 65-wide slot)
        nc.vector.tensor_copy(
            vp[:].rearrange("p (a e) -> p a e", e=65)[:, :, 64:65],
            ones_wide[:].unsqueeze(2),
        )

        with tc.tile_pool(name="pp", bufs=3, space="PSUM") as ppool:
            # warm the PE (HAM ramps on ~3.4us of activity) while
            # the first weight/input DMAs are still landing; these
            # depend only on the on-chip ones constant
            warm = ppool.tile([1, 512], F32, tag="warm", name="warm")
            for _ in range(5):
                nc.tensor.matmul(
                    warm[:], ones_f32[0:1, 0:1], ones_f32[0:1, :],
                    start=True, stop=True,
                )

            # ---- K projection ----
            w_sb = load_w(wkT, ppool, split=2)
            for rb in range(S // XW):
                x_sb = load_x(kT, rb, ppool, split=(2 if rb == 0 else 1))
                if rb == 0:
                    # deferred consts: not needed until Q0/V wrap up
                    nc.sync.dma_start(
                        maskB[:], maskf[:].to_broadcast([128, SQ])
                    )
                    nc.vector.tensor_copy(maskB[0:1, 0:1], maskB[0:1, 0:1])
                    nc.sync.dma_start(bv_sb[:], bvr[:])
                    nc.sync.dma_start(bo_sb[:], bor[:])
                for fcg in range(FCG):
                    ps = ppool.tile([128, XW], F32, tag="ps")
                    for ic in range(IC):
                        nc.tensor.matmul(
                            ps[:],
                            w_sb[:, ic * FG + fcg * 128: ic * FG + fcg * 128 + 128],
                            x_sb[:, ic * XW:(ic + 1) * XW],
                            start=(ic == 0),
                            stop=(ic == IC - 1),
                        )
                    nc.vector.tensor_scalar_add(
                        kt[:, fcg * S + rb * XW: fcg * S + (rb + 1) * XW],
                        ps[:],
                        bkc_sb[:, fcg:fcg + 1],
                    )

            nc.sync.dma_start(
                wo_sb[:].rearrange("p (f o) -> p f o", f=FCG),
                woT[:, :].rearrange("(f p) o -> p f o", p=128),
            )
            nc.sync.dma_start(
                wq_sb[:].rearrange("p (i f) -> p i f", i=IC),
                wqT[:, :].rearrange("(i p) f -> p i f", p=128),
            )

            # ---- V projection + ones column ----
            w_sb = load_w(wvT, ppool)
            NRC = XW // 128
            for rb in range(S // XW):
                x_sb = load_x(vT, rb, ppool)
                for rcl in range(NRC):
                    rc = rb * NRC + rcl
                    ps = ppool.tile([128, FG], F32, tag="ps")
                    for ic in range(IC):
                        nc.tensor.matmul(
                            ps[:],
                            x_sb[:, ic * XW + rcl * 128: ic * XW + rcl * 128 + 128],
                            w_sb[:, ic * FG:(ic + 1) * FG],
                            start=(ic == 0),
                            stop=False,
                        )
                    nc.tensor.matmul(
                        ps[:],
                        ones_row[0:1, 0:128],
                        bv_sb[0:1, 0:FG],
                        start=False,
                        stop=True,
                    )
                    # split the 8 head-slot copies between DVE and the
                    # (idle during projections) scalar engine
                    for hl in range(HPG):
                        dst = vp[:, rc * WV + 65 * hl: rc * WV + 65 * hl + 64]
                        src_ = ps[:, hl * 64:(hl + 1) * 64]
                        if hl % 2 == 0:
                            nc.vector.tensor_copy(dst, src_)
                        else:
                            nc.scalar.activation(dst, src_, AF.Copy)

            # ---- Q projection for the first query block only; the
            # rest pipelines into the attention loop as PE fillers ----
            for op in qproj_closures(0, ppool, "ps", touch_pool=ppool):
                op()

        # ---- attention (software-pipelined vs ScalarE exp) with the
        # output projection of the previous query block interleaved into
        # the PE gaps, so the PE never idles and HAM stays at full clock.
        # PSUM: sp 2 tags x 2 banks each + pv 2 + oproj/qproj 2 = 8 banks.
        with tc.tile_pool(name="sps", bufs=2, space="PSUM") as spool, \
             tc.tile_pool(name="pvp", bufs=2, space="PSUM") as pvpool, \
             tc.tile_pool(name="ops", bufs=2, space="PSUM") as opool, \
             tc.tile_pool(name="esb", bufs=6) as epool, \
             tc.tile_pool(name="nrm", bufs=2) as npool, \
             tc.tile_pool(name="osb", bufs=2) as ospool, \
             tc.tile_pool(name="pvs", bufs=2) as pvspool:

            def outproj_closures(sqb):
                # out[rows of query block sqb] = h_tile.T @ wo (+ bo),
                # split into single-matmul closures for interleaving
                ops = []
                OB = 512
                for rcl in range(SQB // 128):
                    rc = sqb * (SQB // 128) + rcl
                    for ob in range(HID // OB):
                        st = {}

                        def mk(fc, rc=rc, ob=ob, st=st):
                            def run():
                                if fc == 0:
                                    st["ps"] = opool.tile(
                                        [128, OB], F32, tag="ops", name="ops"
                                    )
                                nc.tensor.matmul(
                                    st["ps"][:],
                                    h_tile[:, fc * SQ + rc * 128:
                                           fc * SQ + rc * 128 + 128],
                                    wo_sb[:, fc * HID + ob * OB:
                                          fc * HID + (ob + 1) * OB],
                                    start=(fc == 0),
                                    stop=False,
                                )
                            return run

                        def fin(rc=rc, ob=ob, st=st):
                            nc.tensor.matmul(
                                st["ps"][:],
                                ones_row[0:1, 0:128],
                                bo_sb[0:1, ob * OB:(ob + 1) * OB],
                                start=False,
                                stop=True,
                            )
                            o_sb = ospool.tile([128, OB], F32, tag="o")
                            nc.vector.tensor_copy(o_sb[:], st["ps"][:])
                            nc.sync.dma_start(
                                out[rc * 128:(rc + 1) * 128,
                                    ob * OB:(ob + 1) * OB],
                                o_sb[:],
                            )

                        ops.extend([mk(fc) for fc in range(FCG)])
                        ops.append(fin)
                return ops

            def emit_scores(sqb_, hp_, kc):
                # heads 2hp (rows 0-63) / 2hp+1 (rows 64-127) use
                # disjoint PE row groups; both land in one [128, 1024]
                # 2-bank sp tile (head h at bank h) so a single strided
                # exp covers them
                sp = spool.tile([128, 1024], F32, tag="sp",
                                name="sp")
                kslc = slice(hp_ * S + kc * 128,
                             hp_ * S + kc * 128 + 128)
                for h in range(2):
                    nc.tensor.matmul(
                        sp[:, h * 512:h * 512 + SQB],
                        kt[64 * h:64 * h + 64, kslc],
                        qt[64 * h:64 * h + 64,
                           hp_ * SQ + sqb_ * SQB: hp_ * SQ + (sqb_ + 1) * SQB],
                        start=True, stop=True,
                    )
                return sp

            sp_carry = None
            for sqb in range(nsqb):
                qfill = (qproj_closures(sqb + 1, opool, "ops")
                         if sqb + 1 < nsqb else [])
                ofill = outproj_closures(sqb - 1) if sqb > 0 else []
                qi = oi = 0
                for hp in range(FCG):
                    pvs = [
                        pvpool.tile([65, 512], F32, tag="pv", name="pv")
                        for _ in range(2)
                    ]
                    # the score pipeline carries across hp/sqb
                    # boundaries so ScalarE never drains
                    sp_cur = (sp_carry if sp_carry is not None
                              else emit_scores(sqb, hp, 0))
                    sp_carry = None
                    for kc in range(NKC):
                        if kc + 1 < NKC:
                            sp_nxt = emit_scores(sqb, hp, kc + 1)
                        elif hp + 1 < FCG:
                            sp_nxt = emit_scores(sqb, hp + 1, 0)
                        elif sqb + 1 < nsqb:
                            sp_nxt = emit_scores(sqb + 1, 0, 0)
                        else:
                            sp_nxt = None
                        es = epool.tile([128, 2 * SQB], BF16, tag="es")
                        if sqb == nsqb - 1:
                            # last block: no next-block Q fillers exist, so
                            # the kc loop tends to run exp-bound; split the
                            # exp per head so PV(h0) waits only half an exp
                            for h in range(2):
                                nc.scalar.activation(
                                    es[:, h * SQB:(h + 1) * SQB],
                                    sp_cur[:, h * 512:h * 512 + SQB],
                                    AF.Exp,
                                )
                        else:
                            nc.scalar.activation(
                                es[:].rearrange("p (h w) -> p h w", h=2),
                                sp_cur[:].rearrange(
                                    "p (h w) -> p h w", h=2)[:, :, 0:SQB],
                                AF.Exp,
                            )
                        for h in range(2):
                            hl = 2 * hp + h
                            nc.tensor.matmul(
                                pvs[h][:, 0:SQB],
                                vp[:, kc * WV + 65 * hl:
                                   kc * WV + 65 * hl + 65],
                                es[:, h * SQB:(h + 1) * SQB],
                                start=(kc == 0),
                                stop=(kc == NKC - 1),
                            )
                        # filler ops per kc: Q projection of the next
                        # block, then the output projection of the
                        # previous block (whose h_tile only settles
                        # during hp 0). Fillers are PACED evenly over
                        # the block instead of consumed ASAP -- without
                        # them the kc loop runs exp-bound (~820ns of PE
                        # work vs ~1040ns exp turnaround), so exhausting
                        # them early leaves the last head-pairs stalling
                        # on ScalarE.
                        # no fillers in the last two kc: keeps the DVE
                        # queue clear so the tail's PSUM-freeing copies
                        # run immediately and the next head-pair's PV
                        # isn't gated on a late pv bank
                        if kc < NKC - 2:
                            sidx = hp * (NKC - 2) + kc + 1
                            nslots = FCG * (NKC - 2)
                            qtgt = min(len(qfill),
                                       -(-len(qfill) * sidx // nslots))
                            while qi < qtgt:
                                qfill[qi]()
                                qi += 1
                            if hp >= 1:
                                oidx = (hp - 1) * (NKC - 2) + kc + 1
                                oslots = (FCG - 1) * (NKC - 2)
                                otgt = min(len(ofill),
                                           -(-len(ofill) * oidx // oslots))
                                while oi < otgt:
                                    ofill[oi]()
                                    oi += 1
                        sp_cur = sp_nxt
                    sp_carry = sp_cur

                    for h in range(2):
                        po = 64 * h
                        # copy PSUM->SBUF immediately to free the bank,
                        # then normalize off the SBUF copy. The [1,SQB]
                        # denominator row is reshaped to [64, SQB/64] via
                        # a DRAM bounce so the reciprocal runs on 64 DVE
                        # lanes instead of one (3.3us -> ~0.1us).
                        pv_sb = pvspool.tile([65, SQB], F32, tag="pvsb")
                        nc.vector.tensor_copy(pv_sb[:], pvs[h][:, 0:SQB])
                        rd = dbounce.tile([1, SQB], F32, tag="rd")
                        nc.sync.dma_start(rd[:], pv_sb[64:65, :])
                        d64 = npool.tile([64, SQB // 64], F32, tag="d64")
                        nc.sync.dma_start(
                            d64[:],
                            rd[:].rearrange("o (p j) -> (o p) j", p=64),
                        )
                        r64 = npool.tile([64, SQB // 64], F32, tag="r64")
                        nc.vector.reciprocal(r64[:], d64[:])
                        rd2 = dbounce.tile([64, SQB // 64], F32, tag="rd2")
                        nc.sync.dma_start(rd2[:], r64[:])
                        recipB = npool.tile([64, SQB], F32, tag="recipB")
                        nc.sync.dma_start(
                            recipB[:],
                            rd2[:].rearrange("p j -> (p j)")
                            .unsqueeze(0).to_broadcast([64, SQB]),
                        )
                        nc.vector.tensor_copy(
                            recipB[0:1, 0:1], recipB[0:1, 0:1]
                        )
                        nc.vector.tensor_mul(
                            h_tile[po:po + 64, hp * SQ + sqb * SQB:
                                   hp * SQ + (sqb + 1) * SQB],
                            pv_sb[0:64, :],
                            recipB[:],
                        )
                # drain any leftover fillers before the next query block
                while qi < len(qfill):
                    qfill[qi]()
                    qi += 1
                while oi < len(ofill):
                    ofill[oi]()
                    oi += 1

        # ---- output projection for the last query block: fc-major in
        # its own PSUM pool, so the fc0-2 matmuls (whose h_tile
        # regions are long since ready) execute while the last
        # head-pair's normalization chain drains ----
        with tc.tile_pool(name="fps", bufs=1, space="PSUM") as fpool, \
             tc.tile_pool(name="fosb", bufs=4) as fospool:
            OB = 512
            NT = (SQB // 128) * (HID // OB)  # 6 tiles
            pss = [fpool.tile([128, OB], F32, tag=f"fps{t}", name=f"fps{t}")
                   for t in range(NT)]
            for fc in range(FCG):
                for t in range(NT):
                    rcl, ob = divmod(t, HID // OB)
                    rc = (nsqb - 1) * (SQB // 128) + rcl
                    nc.tensor.matmul(
                        pss[t][:],
                        h_tile[:, fc * SQ + rc * 128: fc * SQ + rc * 128 + 128],
                        wo_sb[:, fc * HID + ob * OB: fc * HID + (ob + 1) * OB],
                        start=(fc == 0),
                        stop=False,
                    )
            for t in range(NT):
                rcl, ob = divmod(t, HID // OB)
                rc = (nsqb - 1) * (SQB // 128) + rcl
                nc.tensor.matmul(
                    pss[t][:],
                    ones_row[0:1, 0:128],
                    bo_sb[0:1, ob * OB:(ob + 1) * OB],
                    start=False,
                    stop=True,
                )
                o_sb = fospool.tile([128, OB], F32, tag="fo")
                nc.vector.tensor_copy(o_sb[:], pss[t][:])
                nc.sync.dma_start(
                    out[rc * 128:(rc + 1) * 128, ob * OB:(ob + 1) * OB],
                    o_sb[:],
                )
    return nc


_compiled = {}


def _get_nc(nsqb):
    if nsqb not in _compiled:
        nc = bacc.Bacc(
            "TRN2", target_bir_lowering=False, debug=False, num_devices=N_CORES
        )
        build(nc, nsqb=nsqb)
        nc.compile()
        _compiled[nsqb] = nc
    return _compiled[nsqb]


def _bf16(a):
    import ml_dtypes

    return np.ascontiguousarray(a).astype(ml_dtypes.bfloat16)


def _compact(mask):
    """Per batch: indices of unmasked rows; padded query count covers
    n_unmasked + 1 (representative masked row) for every batch."""
    idxs = [np.nonzero(mask[b] != 0)[0] for b in range(B)]
    need = max(len(ix) + 1 for ix in idxs)
    nsqb = max(1, -(-need // SQB))
    return idxs, nsqb


def prepare_in_maps(inputs, idxs, nsqb):
    SQ = nsqb * SQB
    q = np.asarray(inputs["q"], dtype=np.float32)
    k = np.asarray(inputs["k"], dtype=np.float32)
    v = np.asarray(inputs["v"], dtype=np.float32)
    f32 = np.float32
    scale = f32(1.0 / np.sqrt(DH))

    qT_b, maskf_b = [], []
    for b in range(B):
        ix = idxs[b]
        qc = np.zeros((SQ, HID), f32)
        qc[: len(ix)] = q[b][ix]
        qT_b.append(_bf16(qc.T))
        mf = np.zeros((1, SQ), f32)
        mf[0, : len(ix)] = scale
        maskf_b.append(mf)
    kT_b = [_bf16(k[b].T) for b in range(B)]
    vT_b = [_bf16(v[b].T) for b in range(B)]

    wqT = np.asarray(inputs["wq"], f32).T  # [in, out]
    wkT = np.asarray(inputs["wk"], f32).T
    wvT = np.asarray(inputs["wv"], f32).T
    woT = np.asarray(inputs["wo"], f32).T  # [in(=h dims), out]
    bq = np.asarray(inputs["bq"], f32).reshape(1, HID)
    bk = np.asarray(inputs["bk"], f32).reshape(1, HID)
    bv = np.asarray(inputs["bv"], f32).reshape(1, HID)
    bo = np.asarray(inputs["bo"], f32).reshape(1, HID)

    per_hh = []
    for hh in range(2):
        cols = slice(hh * FG, (hh + 1) * FG)
        per_hh.append({
            "wqT": _bf16(wqT[:, cols]),
            "wkT": _bf16(wkT[:, cols]),
            "wvT": _bf16(wvT[:, cols]),
            "woT": _bf16(woT[cols, :]),
            "bqc": np.ascontiguousarray(
                bq[0, cols].reshape(FCG, 128).T.astype(f32)),
            "bkc": np.ascontiguousarray(
                bk[0, cols].reshape(FCG, 128).T.astype(f32)),
            "bvr": _bf16(bv[:, cols]),
            # partials are summed on the host: only hh==0 contributes bo
            "bor": _bf16(bo if hh == 0 else np.zeros_like(bo)),
        })

    in_maps = []
    for c in range(N_CORES):
        b, hh = c // 2, c % 2
        m = dict(per_hh[hh])
        m["qT"] = qT_b[b]
        m["kT"] = kT_b[b]
        m["vT"] = vT_b[b]
        m["maskf"] = maskf_b[b]
        in_maps.append(m)
    return in_maps


def kernel(q, k, v, mask, wq, bq, wk, bk, wv, bv, wo, bo):
    mask = np.asarray(mask)
    idxs, nsqb = _compact(mask)
    nc = _get_nc(nsqb)
    in_maps = prepare_in_maps(dict(
        q=q, k=k, v=v, wq=wq, bq=bq, wk=wk, bk=bk,
        wv=wv, bv=bv, wo=wo, bo=bo,
    ), idxs, nsqb)

    res = run_bass_kernel_spmd(nc, in_maps, list(range(N_CORES)), trace=TRACE)
    LAST_RESULTS[0] = res

    out = np.empty((B, S, HID), dtype=np.float32)
    for b in range(B):
        full = res.results[2 * b]["out"] + res.results[2 * b + 1]["out"]
        ix = idxs[b]
        out[b][ix] = full[: len(ix)]
        if len(ix) < S:
            # every masked row equals the representative uniform-softmax row
            out[b][mask[b] == 0] = full[len(ix)]
    return out
